# revision 49
# baseline (speedup 1.0000x reference)
"""Trainium2 Bass kernel for 4-layer bidirectional GRU (H=128, T=200) + MLP head.

Data-parallel over the 400 flattened sequences -> 50 per core on 8 cores.
Layout: 128 partitions = hidden unit, free dim = batch slots [fwd 50 | bwd 50].

Latency-optimized scan (2.92ms -> 1.97ms vs the v1 baseline):
  - Gate pre-activations live in PSUM, written chunk-ahead (CT=5 steps) by
    precompute matmuls: regions R, Zbar, N(gi_n), Q(bhh_n broadcast), one
    bank each, double-buffered = 8 banks. Biases ride K=2 masked
    ones-matmuls (row0 -> fwd cols, row1 -> bwd cols). No PSUM prefill
    matmul in the scan and no scalar-engine evictions at all.
  - The z gate is negated end-to-end (weights+biases), so region Zbar holds
    the zbar preactivation and ONE sigmoid covers [r|zbar]; this kills both
    the second ACT op and a Tile wait-coalescing artifact that serialized
    it onto the chain. h' = zbar*n + (h - zbar*h).
  - h' = a + b (a = zbar*n, b = z*h) is exploited in the recurrence:
    Whh@h' is accumulated as separate matmuls on a and b, so the h'-add
    leaves the critical chain; only the a-matmuls (after tanh) gate the
    next sigmoid. tmp = Q[t]*r on DVE; an identity matmul accumulates tmp
    onto the N region (PSUM f32 add); tanh reads PSUM directly.
  - Layer 3 runs forward-only after step 0 (step 0 is bidirectional with
    h0=0, which yields exactly the backward output at the last timestep
    that the readout needs).
"""

import os
import sys

import numpy as np

_REPO = "/opt/trn_rl_repo"
if _REPO not in sys.path:
    sys.path.insert(0, _REPO)

B, KSEQ, T = 4, 100, 200
H = 128
L = 4
OUT = 8
NCORES = 8
N = B * KSEQ              # 400 sequences
NB = N // NCORES          # 50 per core
CT = 5                    # timesteps per PSUM chunk (CT*2*NB = 500 <= 512)
F16 = "float16"

_CACHE = {}

# PSUM region offsets (f32 elements within the 4-bank chunk tile)
R_OFF, Z_OFF, N_OFF, Q_OFF = 0, 512, 1024, 1536


def _build_program(t_len=T, nb=NB, ct=CT):
    import concourse.bacc as bacc
    import concourse.mybir as mybir
    import concourse.tile as tile
    from contextlib import ExitStack

    f32 = mybir.dt.float32
    f16 = mybir.dt.float16

    nch = t_len // ct           # 40 chunks
    W = 2 * nb                  # 100: [fwd 50 | bwd 50]

    nc = bacc.Bacc("TRN2", target_bir_lowering=False, debug=False,
                   num_devices=NCORES)

    # ---- DRAM I/O ----
    dx0f = nc.dram_tensor("x0f", (2, t_len * nb), f16, kind="ExternalInput").ap()
    dx0r = nc.dram_tensor("x0r", (2, t_len * nb), f16, kind="ExternalInput").ap()
    dw0 = nc.dram_tensor("w0", (2, 6 * H), f16, kind="ExternalInput").ap()
    dwih = nc.dram_tensor("wihT", (36, H, H), f16, kind="ExternalInput").ap()
    dwhh = nc.dram_tensor("whhT", (24, H, H), f16, kind="ExternalInput").ap()
    dbrows = nc.dram_tensor("brows", (2, 13 * H), f16, kind="ExternalInput").ap()
    dmask = nc.dram_tensor("mask", (2, ct * W), f16, kind="ExternalInput").ap()
    dbiasn = nc.dram_tensor("biasn", (H, 3 * ct * W), f16,
                            kind="ExternalInput").ap()
    dident = nc.dram_tensor("ident", (H, H), f16, kind="ExternalInput").ap()
    dw1 = nc.dram_tensor("w1T", (2, H, H), f16, kind="ExternalInput").ap()
    db1 = nc.dram_tensor("b1col", (H, 1), f32, kind="ExternalInput").ap()
    dw2 = nc.dram_tensor("w2T", (H, OUT), f32, kind="ExternalInput").ap()
    db2 = nc.dram_tensor("b2col", (OUT, 1), f32, kind="ExternalInput").ap()
    dout = nc.dram_tensor("out", (OUT, nb), f32, kind="ExternalOutput").ap()

    with tile.TileContext(nc) as tc, ExitStack() as ctx:
        cpool = ctx.enter_context(tc.tile_pool(name="consts", bufs=1))
        xpool = ctx.enter_context(tc.tile_pool(name="xcat", bufs=1))
        spool = ctx.enter_context(tc.tile_pool(name="scratch", bufs=3))
        hpool = ctx.enter_context(tc.tile_pool(name="hstate", bufs=2))

        # ---- constants / weights to SBUF ----
        w0_sb = cpool.tile([2, 6 * H], f16)
        nc.sync.dma_start(w0_sb[:], dw0)
        wih_sb = cpool.tile([H, 36 * H], f16)
        nc.sync.dma_start(wih_sb[:].rearrange("p (i c) -> p i c", c=H),
                          dwih.rearrange("i p c -> p i c"))
        whh_sb = cpool.tile([H, 24 * H], f16)
        nc.sync.dma_start(whh_sb[:].rearrange("p (i c) -> p i c", c=H),
                          dwhh.rearrange("i p c -> p i c"))
        brows_sb = cpool.tile([2, 13 * H], f16)
        nc.sync.dma_start(brows_sb[:], dbrows)
        mask_sb = cpool.tile([2, ct * W], f16)
        nc.sync.dma_start(mask_sb[:], dmask)
        biasn_sb = cpool.tile([H, 3 * ct * W], f16)
        nc.sync.dma_start(biasn_sb[:], dbiasn)
        id_sb = cpool.tile([H, H], f16)
        nc.sync.dma_start(id_sb[:], dident)
        w1_sb = cpool.tile([H, 2 * H], f16)
        nc.sync.dma_start(w1_sb[:].rearrange("p (i c) -> p i c", c=H),
                          dw1.rearrange("i p c -> p i c"))
        b1_sb = cpool.tile([H, 1], f32)
        nc.sync.dma_start(b1_sb[:], db1)
        w2_sb = cpool.tile([H, OUT], f32)
        nc.sync.dma_start(w2_sb[:], dw2)
        b2_sb = cpool.tile([OUT, 1], f32)
        nc.sync.dma_start(b2_sb[:], db2)
        xA = xpool.tile([H, t_len * W], f16, tag="xA")
        xB = xpool.tile([H, t_len * W], f16, tag="xB")

        def wih_t(l, d, g, k):  # layers 1..3
            i = (((l - 1) * 2 + d) * 3 + g) * 2 + k
            return wih_sb[:, i * H:(i + 1) * H]

        def whh_t(l, d, g):
            i = (l * 2 + d) * 3 + g
            return whh_sb[:, i * H:(i + 1) * H]

        # brows index map: layers 1..3 regions (r,z,n,q) -> 12, layer0 q -> 12
        def b_idx(l, reg):
            if l == 0:
                return 12
            return (l - 1) * 4 + reg

        def mrow(idx):
            return brows_sb[:, idx * H:(idx + 1) * H]

        mask3 = mask_sb[:].rearrange("p (t w) -> p t w", w=W)

        # ------------------------------------------------------------------
        def reg3(psum, reg):
            """(p, ct, W) view of PSUM region reg (regions are bank-padded)."""
            return psum[:, reg * 512:reg * 512 + ct * W].rearrange(
                "p (t w) -> p t w", w=W)

        def make_prefill(l, c, psum, x_in, x0f_sb, x0r_sb, dirs):
            """Returns a list of thunks; each emits 1 matmul for chunk c.
            Per region the FIRST thunk carries start=True (whole-bank clear);
            bias matmuls go LAST (their WAR deps are the freshest)."""
            thunks = []

            def bias_mm(reg, both_dirs, start):
                rhs = mask3 if both_dirs else mask3[:, :, 0:nb]
                r3 = reg3(psum, reg)
                out = r3 if both_dirs else r3[:, :, 0:nb]
                bi = b_idx(l, reg)
                thunks.append(lambda out=out, bi=bi, rhs=rhs, st=start:
                              nc.tensor.matmul(out, mrow(bi), rhs,
                                               start=st, stop=False))

            s0 = c * ct
            hi = t_len - 1 - s0
            lo = hi - ct
            asc = slice(s0, s0 + ct)
            dsc = slice(hi, lo if lo >= 0 else None, -1)

            if l == 0:
                # layer 0: K=2 gi matmuls carry weights+biases (w0 rows)
                for reg, g in ((0, 0), (1, 1), (2, 2)):
                    for d in dirs:
                        src = x0f_sb if d == 0 else x0r_sb
                        rhs = src[:, c * ct * nb:(c + 1) * ct * nb].rearrange(
                            "p (t n) -> p t n", n=nb)
                        out = reg3(psum, reg)[:, :, d * nb:(d + 1) * nb]
                        lhsT = w0_sb[:, (d * 3 + g) * H:(d * 3 + g + 1) * H]
                        st = (d == dirs[0])
                        thunks.append(lambda out=out, lhsT=lhsT, rhs=rhs, st=st:
                                      nc.tensor.matmul(out, lhsT, rhs,
                                                       start=st, stop=False))
                bias_mm(3, len(dirs) == 2, True)   # q = bhh_n broadcast
            else:
                x3 = x_in[:].rearrange("p (t w) -> p t w", w=W)
                for reg, g in ((0, 0), (1, 1), (2, 2)):
                    first = True
                    for d in dirs:
                        r0 = x3[:, asc if d == 0 else dsc, 0:nb]
                        r1 = x3[:, dsc if d == 0 else asc, nb:W]
                        out = reg3(psum, reg)[:, :, d * nb:(d + 1) * nb]
                        for k, rr in ((0, r0), (1, r1)):
                            thunks.append(
                                lambda out=out, lhsT=wih_t(l, d, g, k), rr=rr,
                                st=first: nc.tensor.matmul(out, lhsT, rr,
                                                           start=st,
                                                           stop=False))
                            first = False
                bias_mm(3, len(dirs) == 2, True)
                for reg in (0, 1):
                    bias_mm(reg, len(dirs) == 2, False)
                # N-region bias via a DVE in-place PSUM add (PE relief; the
                # gi matmuls above already set has_written, so the ident
                # matmul still accumulates on top)
                wb = W if len(dirs) == 2 else nb
                nf = reg3(psum, 2)[:, :, 0:wb]
                bn = biasn_sb[:].rearrange("p (l t w) -> p l t w", l=3, w=W)[
                    :, l - 1, :, 0:wb]
                thunks.append(lambda nf=nf, bn=bn: nc.vector.tensor_tensor(
                    nf, nf, bn, op=mybir.AluOpType.add))
            return thunks

        # ------------------------------------------------------------------
        def scan_step(l, s, psum, tl, prev, x_out, pending, nthunk, dirs):
            """One GRU step. prev = (h_ap, a_ap, b_ap); a/b are None on the
            first step (h0 = 0). Returns (h_new, a_ap, b_ap).

            Recurrent trick: h = a + b, so Whh@h is accumulated as two
            matmuls on a and b separately. The b-matmuls (b = z*h, ready
            right after the sigmoid) run inside the tanh window; only the
            a-matmuls sit on the critical chain after tanh."""
            h_prev, a_prev, b_prev = prev
            w = len(dirs) * nb
            Nt = reg3(psum, 2)[:, tl, 0:w]
            Qt = reg3(psum, 3)[:, tl, 0:w]
            # (p, 2, w) view spanning the R and Zbar banks at step tl
            rzb_in = psum[:, 0:1024].rearrange("p (r x) -> p r x", r=2)[
                :, :, tl * W:tl * W + w]

            # Recurrent matmuls. z-gate weights are pre-negated host-side
            # (zbar preactivation). b-group first (b is ready earliest),
            # a-group second ordered q,zbar,r so the sigmoid's (coarsened)
            # wait lands exactly on its true gating matmul.
            if a_prev is None:
                for reg, g in ((0, 0), (1, 1), (3, 2)):
                    for j, d in enumerate(dirs):
                        nc.tensor.matmul(
                            reg3(psum, reg)[:, tl, d * nb:(d + 1) * nb],
                            whh_t(l, d, g), h_prev[:, j * nb:(j + 1) * nb],
                            start=False, stop=(d == dirs[-1]))
            else:
                for reg, g in ((0, 0), (1, 1)):
                    for j, d in enumerate(dirs):
                        nc.tensor.matmul(
                            reg3(psum, reg)[:, tl, d * nb:(d + 1) * nb],
                            whh_t(l, d, g), b_prev[:, j * nb:(j + 1) * nb],
                            start=False, stop=False)
                for reg, g in ((1, 1), (0, 0)):
                    for j, d in reversed(list(enumerate(dirs))):
                        nc.tensor.matmul(
                            reg3(psum, reg)[:, tl, d * nb:(d + 1) * nb],
                            whh_t(l, d, g), a_prev[:, j * nb:(j + 1) * nb],
                            start=False, stop=(reg == 0 and d == dirs[0]))
                # q on h' directly (h' is ready before these reach the PE):
                # 2 matmuls instead of 4, same issue depth ahead of sigma
                for j, d in enumerate(dirs):
                    nc.tensor.matmul(
                        reg3(psum, 3)[:, tl, d * nb:(d + 1) * nb],
                        whh_t(l, d, 2), h_prev[:, j * nb:(j + 1) * nb],
                        start=False, stop=(d == dirs[-1]))

            # precompute matmuls for the next chunk ride in the PE idle windows
            for _ in range(nthunk):
                if pending:
                    pending.pop(0)()

            # one sigmoid covers [r | zbar] (zbar preactivation is negated z)
            rz_sb = spool.tile([H, 2 * W], f16, tag="rz_sb")
            nc.scalar.activation(
                rz_sb[:].rearrange("p (r w) -> p r w", r=2)[:, :, 0:w],
                rzb_in, mybir.ActivationFunctionType.Sigmoid)
            r_ap = rz_sb[:, 0:w]
            zb_ap = rz_sb[:, W:W + w]

            tmp = spool.tile([H, W], f16, tag="tmp")
            nc.vector.tensor_tensor(tmp[:, 0:w], Qt, r_ap,
                                    op=mybir.AluOpType.mult)
            # N[t] += I @ tmp  (PSUM f32 accumulate on the tensor engine)
            nc.tensor.matmul(Nt, id_sb[:], tmp[:, 0:w], start=False, stop=True)

            n_sb = spool.tile([H, W], f16, tag="n_sb")
            nc.scalar.activation(n_sb[:, 0:w], Nt,
                                 mybir.ActivationFunctionType.Tanh)

            u_sb = spool.tile([H, W], f16, tag="u_sb")
            nc.vector.tensor_tensor(u_sb[:, 0:w], zb_ap, h_prev,
                                    op=mybir.AluOpType.mult)
            b_sb = spool.tile([H, W], f16, tag="b_sb")
            nc.vector.tensor_tensor(b_sb[:, 0:w], h_prev, u_sb[:, 0:w],
                                    op=mybir.AluOpType.subtract)
            a_sb = spool.tile([H, W], f16, tag="a_sb")
            nc.vector.tensor_tensor(a_sb[:, 0:w], zb_ap, n_sb[:, 0:w],
                                    op=mybir.AluOpType.mult)
            h_new = x_out[:].rearrange("p (t w) -> p t w", w=W)[:, s, 0:w]
            nc.vector.tensor_tensor(h_new, a_sb[:, 0:w], b_sb[:, 0:w],
                                    op=mybir.AluOpType.add)
            return h_new, a_sb[:, 0:w], b_sb[:, 0:w]

        # ------------------------------------------------------------------
        def run_layer(l, x_in, x_out, pscan, x0f_sb=None, x0r_sb=None,
                      fwd_only_after0=False):
            both = (0, 1)
            fwd = (0,)

            def chunk_dirs(c):
                if fwd_only_after0 and c > 0:
                    return fwd
                return both

            tiles = {}

            def prefill(c):
                psum = pscan.tile([H, 4 * 512], f32, tag="pscan")
                tiles[c] = psum
                return make_prefill(l, c, psum, x_in, x0f_sb, x0r_sb,
                                    chunk_dirs(c))

            # chunk 0 fully prefilled up front; chunk c+1 rides chunk c's steps
            for th in prefill(0):
                th()
            h0 = hpool.tile([H, W], f16, tag="h0")
            nc.vector.memset(h0[:], 0.0)
            prev = (h0[:], None, None)
            for c in range(nch):
                pending = prefill(c + 1) if c + 1 < nch else []
                # spread pops evenly; the remainder lands on the LAST steps
                # so the bias matmuls (freshest WAR deps, last in the list)
                # pop as late as possible
                npend = len(pending)
                counts = [npend // ct] * ct
                for i in range(npend % ct):
                    counts[ct - 1 - i] += 1
                for tl in range(ct):
                    s = c * ct + tl
                    dirs = both if (s == 0 or not fwd_only_after0) else fwd
                    prev = scan_step(l, s, tiles[c], tl, prev, x_out, pending,
                                     counts[tl], dirs)
                    if s == 0 and fwd_only_after0:
                        prev = tuple(p[:, 0:nb] for p in prev)
                del tiles[c]

        # ---------------- layers ----------------
        with tc.tile_pool(name="l0feed", bufs=1) as fpool, \
             tc.tile_pool(name="pscan", bufs=2, space="PSUM") as pscan:
            x0f_sb = fpool.tile([2, t_len * nb], f16)
            nc.sync.dma_start(x0f_sb[:], dx0f)
            x0r_sb = fpool.tile([2, t_len * nb], f16)
            nc.sync.dma_start(x0r_sb[:], dx0r)

            run_layer(0, None, xA, pscan, x0f_sb, x0r_sb)
            run_layer(1, xA, xB, pscan)
            run_layer(2, xB, xA, pscan)
            run_layer(3, xA, xB, pscan, fwd_only_after0=True)

        # ---------------- MLP head ----------------
        xB3 = xB[:].rearrange("p (t w) -> p t w", w=W)
        hf = xB3[:, t_len - 1, 0:nb]
        hb = xB3[:, 0, nb:W]
        with tc.tile_pool(name="phead", bufs=1, space="PSUM") as php:
            ph1 = php.tile([H, nb], f32)
            nc.tensor.matmul(ph1[:], w1_sb[:, 0:H], hf,
                             start=True, stop=False)
            nc.tensor.matmul(ph1[:], w1_sb[:, H:2 * H], hb,
                             start=False, stop=True)
            h1p = spool.tile([H, nb], f32, tag="h1p")
            nc.scalar.activation(h1p[:], ph1[:],
                                 mybir.ActivationFunctionType.Identity,
                                 bias=b1_sb[:])
            h1 = spool.tile([H, nb], f32, tag="h1")
            nc.vector.scalar_tensor_tensor(
                h1[:], h1p[:], 0.2, h1p[:],
                op0=mybir.AluOpType.mult, op1=mybir.AluOpType.max)
            po = php.tile([OUT, nb], f32)
            nc.tensor.matmul(po[:], w2_sb[:], h1[:], start=True, stop=True)
            o_sb = spool.tile([OUT, nb], f32, tag="o_sb")
            nc.scalar.activation(o_sb[:], po[:],
                                 mybir.ActivationFunctionType.Identity,
                                 bias=b2_sb[:])
            nc.sync.dma_start(dout, o_sb[:])

    nc.compile()
    return nc


def _prep_host(raw, Wih0, Wih, Whh, bih, bhh, W1, b1, W2, b2,
               t_len=T, nb=NB, ct=CT):
    """Host-side weight/layout prep. Returns (shared_inputs, per_core_feeds)."""
    f16 = np.float16
    Wih0 = np.asarray(Wih0, np.float32)
    Wih = np.asarray(Wih, np.float32)
    Whh = np.asarray(Whh, np.float32)
    bih = np.asarray(bih, np.float32)
    bhh = np.asarray(bhh, np.float32)

    # The z gate is computed as zbar = sigmoid(-z_preact): negate every
    # z-path weight/bias (marked "zsign") so PSUM region 1 holds -z_preact.
    def zsign(g):
        return -1.0 if g == 1 else 1.0

    # layer0 lhsT (2, 6*128): row0 weights, row1 combined bias
    w0 = np.zeros((2, 6 * H), np.float32)
    for d in range(2):
        for g in range(3):
            sl = slice(g * H, (g + 1) * H)
            w0[0, (d * 3 + g) * H:(d * 3 + g + 1) * H] = \
                zsign(g) * Wih0[d, sl, 0]
            bb = bih[0, d, sl] + (bhh[0, d, sl] if g < 2 else 0.0)
            w0[1, (d * 3 + g) * H:(d * 3 + g + 1) * H] = zsign(g) * bb

    wihT = np.zeros((36, H, H), np.float32)
    for l in range(1, 4):
        for d in range(2):
            for g in range(3):
                for k in range(2):
                    i = (((l - 1) * 2 + d) * 3 + g) * 2 + k
                    wihT[i] = zsign(g) * Wih[l - 1, d, g * H:(g + 1) * H,
                                             k * H:(k + 1) * H].T
    whhT = np.zeros((24, H, H), np.float32)
    for l in range(4):
        for d in range(2):
            for g in range(3):
                whhT[(l * 2 + d) * 3 + g] = \
                    zsign(g) * Whh[l, d, g * H:(g + 1) * H, :].T

    # bias K=2 lhsT rows (row d = dir-d bias): layers 1..3 x regions
    # (r,zbar,n,q), plus layer0 q
    brows = np.zeros((2, 13 * H), np.float32)
    for l in range(1, 4):
        for d in range(2):
            r = bih[l, d, 0:H] + bhh[l, d, 0:H]
            z = -(bih[l, d, H:2 * H] + bhh[l, d, H:2 * H])
            n = bih[l, d, 2 * H:3 * H]
            q = bhh[l, d, 2 * H:3 * H]
            for reg, v in enumerate((r, z, n, q)):
                i = (l - 1) * 4 + reg
                brows[d, i * H:(i + 1) * H] = v
    for d in range(2):
        brows[d, 12 * H:13 * H] = bhh[0, d, 2 * H:3 * H]

    # mask (2, ct*100): row0 selects fwd cols, row1 selects bwd cols
    mask = np.zeros((2, ct * 2 * nb), np.float32)
    m3 = mask.reshape(2, ct, 2 * nb)
    m3[0, :, 0:nb] = 1.0
    m3[1, :, nb:2 * nb] = 1.0

    # N-region bias pattern for layers 1..3: (H, l, t, [f 50 | b 50])
    biasn = np.zeros((H, 3, ct, 2 * nb), np.float32)
    for l in range(1, 4):
        for d in range(2):
            biasn[:, l - 1, :, d * nb:(d + 1) * nb] = \
                bih[l, d, 2 * H:3 * H][:, None, None]

    shared = {
        "w0": w0.astype(f16),
        "wihT": wihT.astype(f16),
        "whhT": whhT.astype(f16),
        "brows": brows.astype(f16),
        "mask": mask.astype(f16),
        "biasn": biasn.reshape(H, -1).astype(f16),
        "ident": np.eye(H, dtype=f16),
        "w1T": np.stack([np.asarray(W1, np.float32)[:, 0:H].T,
                         np.asarray(W1, np.float32)[:, H:2 * H].T]).astype(f16),
        "b1col": np.asarray(b1, np.float32).reshape(H, 1),
        "w2T": np.asarray(W2, np.float32).T.copy(),
        "b2col": np.asarray(b2, np.float32).reshape(OUT, 1),
    }

    x = np.asarray(raw, np.float32).reshape(N, t_len)
    feeds = []
    for c in range(NCORES):
        xs = x[c * nb:(c + 1) * nb]            # (nb, t)
        x0f = np.ones((2, t_len * nb), np.float32)
        x0f[0] = xs.T.reshape(-1)              # col t*nb+n
        x0r = np.ones((2, t_len * nb), np.float32)
        x0r[0] = xs.T[::-1].reshape(-1)        # col s*nb+n = x[n, t-1-s]
        feeds.append({"x0f": x0f.astype(f16), "x0r": x0r.astype(f16)})
    return shared, feeds


def kernel(raw, Wih0, Wih, Whh, bih, bhh, W1, b1, W2, b2):
    from concourse.bass_utils import run_bass_kernel_spmd

    if "prog" not in _CACHE:
        _CACHE["prog"] = _build_program()
    nc = _CACHE["prog"]

    shared, feeds = _prep_host(raw, Wih0, Wih, Whh, bih, bhh, W1, b1, W2, b2)
    in_maps = [dict(shared, **feeds[c]) for c in range(NCORES)]
    res = run_bass_kernel_spmd(nc, in_maps, list(range(NCORES)),
                               **_CACHE.get("run_kwargs", {}))
    _CACHE["last_results"] = res
    outs = [np.asarray(res.results[c]["out"], np.float32) for c in range(NCORES)]
    full = np.concatenate(outs, axis=1)        # (8, 400)
    return np.ascontiguousarray(full.T).reshape(B, KSEQ, OUT).astype(np.float32)
